# revision 53
# baseline (speedup 1.0000x reference)
"""
Multi-head attention + residual + LayerNorm Trainium2 kernel (8 NeuronCores).

Problem (hardcoded shapes):
    hidden_states [2, 2048, 1024] f32, mask [2, 2048, 2048] int32,
    Wq/Wk/Wv/Wd [1024, 1024] f32, bd/gamma/beta [1024] f32.
    out = LayerNorm(ctx @ Wd.T + bd + hidden_states) with 16 heads, hd=64.

Sharding: pure data parallel. Core c handles batch b = c//4 and query rows
q in [ (c%4)*512, (c%4)*512+512 ).  Each core computes K/V for the full
sequence of its batch (4x redundant), attention + dense + LN for its own
512 rows.  No collectives.

On-device layout is transposed ("feature on partitions") end to end:
    xT   [1024, 2048]  (kv axis rolled so the core's own q rows come first)
    qT_h [64, 512], kT_h [64, 2048]  -> scores sT [kv, q] = kT.T-slices @ qT
    softmax: exp on ScalarE straight out of PSUM (scale=1/8, no max
    subtraction -- scores are O(3) by construction), mask applied as a
    bf16 multiply on VectorE, and the normalizer comes free from an extra
    all-ones column appended to V in the ctx matmul (row 64 of the
    transposed context = sum of masked probs).  Per head only two cheap
    drains run (unnormalized ctxT + sums row); normalization is batched at
    the end: one reciprocal over [16 heads, 512], a selector matmul that
    broadcasts each head pair's reciprocals to [128, 512], one multiply.
Projection work for head pairs 1..7 and v-group 1 is emitted as units
interleaved into the attention loop so the PE stream has no idle bubbles
(keeps the HAM clock gate at 2.4 GHz).  Dense: out[rows, 1024] with
lhsT = ctxT chunks, rhs = Wd.T chunks; add (x + bd) residual (bias folded
on host), LayerNorm on the free dim via bn_stats (gamma/beta passes are
compiled out when they are identity, which the spec's fills guarantee).
"""

import os
import sys
from contextlib import ExitStack

import numpy as np

for _p in ("/opt/trn_rl_repo",):
    if os.path.isdir(_p) and _p not in sys.path:
        sys.path.insert(0, _p)

import ml_dtypes  # noqa: E402

import concourse.bass as bass  # noqa: E402
import concourse.tile as tile  # noqa: E402
from concourse import bacc, mybir  # noqa: E402
from concourse.bass_utils import run_bass_kernel_spmd  # noqa: E402

BF16 = mybir.dt.bfloat16
F32 = mybir.dt.float32
NP_BF16 = ml_dtypes.bfloat16

B, S, H, NH = 2, 2048, 1024, 16
HD = H // NH  # 64
P = 128
NCORES = 8
SQ = S // 4  # 512 query rows per core
FC = H // P  # 8 feature chunks
KC = S // P  # 16 kv chunks
SCALE = 1.0 / float(np.sqrt(HD))
EPS = 1e-6

# Results of the last device run (for test harness introspection)
last_results = None


def _build_program(affine=True):
    nc = bacc.Bacc(
        "TRN2",
        target_bir_lowering=False,
        debug=False,
        enable_asserts=False,
        num_devices=NCORES,
    )

    # Per-core DRAM inputs
    d_xT = nc.dram_tensor("xT", [FC, P, S], BF16, kind="ExternalInput").ap()
    d_wq = nc.dram_tensor("wqT", [FC, P, H], BF16, kind="ExternalInput").ap()
    d_wk = nc.dram_tensor("wkT", [FC, P, H], BF16, kind="ExternalInput").ap()
    d_wv = nc.dram_tensor("wvT", [FC, P, H], BF16, kind="ExternalInput").ap()
    d_wd = nc.dram_tensor("wdT", [FC, P, H], BF16, kind="ExternalInput").ap()
    d_maskT = nc.dram_tensor("maskT", [KC, P, SQ], BF16, kind="ExternalInput").ap()
    d_xres = nc.dram_tensor("xres", [SQ // P, P, H], F32, kind="ExternalInput").ap()
    d_gamma = nc.dram_tensor("gamma", [H], F32, kind="ExternalInput").ap()
    d_beta = nc.dram_tensor("beta", [H], F32, kind="ExternalInput").ap()
    d_sel = nc.dram_tensor("sel", [NH, FC, P], F32, kind="ExternalInput").ap()
    d_out = nc.dram_tensor("out", [SQ // P, P, H], F32, kind="ExternalOutput").ap()

    with tile.TileContext(nc, trace_sim=False) as tc:
        _program(tc, d_xT, d_wq, d_wk, d_wv, d_wd, d_maskT, d_xres, d_gamma,
                 d_beta, d_sel, d_out, affine)

    nc.compile()
    return nc


def _bcast_ap(src_1d, parts):
    """AP that replicates a [n] DRAM vector across `parts` partitions."""
    return bass.AP(
        tensor=src_1d.tensor,
        offset=src_1d.offset,
        ap=[[0, parts]] + list(src_1d.ap),
    )


def _program(ctx_or_tc, *args):
    with ExitStack() as ctx:
        _program_inner(ctx, ctx_or_tc, *args)


def _program_inner(ctx, tc, d_xT, d_wq, d_wk, d_wv, d_wd, d_maskT, d_xres,
                   d_gamma, d_beta, d_sel, d_out, affine):
    from collections import deque
    nc = tc.nc

    # ---------------- pools ----------------
    persist = ctx.enter_context(tc.tile_pool(name="persist", bufs=1))
    ps_mm = ctx.enter_context(tc.tile_pool(name="ps_mm", bufs=2, space="PSUM"))
    ps_s = ctx.enter_context(tc.tile_pool(name="ps_s", bufs=2, space="PSUM"))
    ps_c = ctx.enter_context(tc.tile_pool(name="ps_c", bufs=2, space="PSUM"))

    # ---------------- persistent tiles (split per head-pair / v-group so
    # interleaved projection writes never collide with attention reads) ----
    kT_hp = [persist.tile([P, S], BF16, name=f"kT{hp}") for hp in range(FC)]
    qT_hp = [persist.tile([P, SQ], BF16, name=f"qT{hp}") for hp in range(FC)]
    v_g = [persist.tile([P, KC, 8, HD + 1], BF16, name=f"v{g}") for g in range(2)]
    ctxT_sb = persist.tile([P, FC, SQ], BF16)   # normalized in place per pair
    maskT_sb = persist.tile([P, KC, SQ], BF16)
    sums16 = persist.tile([NH, SQ], F32)

    nc.gpsimd.dma_start(out=maskT_sb, in_=d_maskT.rearrange("c p n -> p c n"))
    for g in range(2):
        nc.vector.memset(v_g[g][:, :, :, HD : HD + 1], 1.0)

    work = ctx.enter_context(tc.tile_pool(name="work", bufs=3))

    # ---------------- projections (emitted as units; the tail of them is
    # interleaved into the attention emission to fill PE gaps) ----------------
    from contextlib import ExitStack as _ES
    proj_ctx = _ES()
    pool_xt = proj_ctx.enter_context(tc.tile_pool(name="proj_xt", bufs=1))
    xT_sb = pool_xt.tile([P, FC, S], BF16)
    pool_w = proj_ctx.enter_context(tc.tile_pool(name="proj_w", bufs=1))
    wq_sb = pool_w.tile([P, FC, H], BF16)
    wk_sb = pool_w.tile([P, FC, H], BF16)
    wv_sb = pool_w.tile([P, FC, H], BF16)
    # interleaved across two HW queues so the first prefix matmuls can
    # start after ~2 small transfers instead of after one big serial load
    eng = [nc.sync, nc.scalar]
    nc.sync.dma_start(out=xT_sb[:, 0, :], in_=d_xT[0])
    nc.scalar.dma_start(out=xT_sb[:, 1, :], in_=d_xT[1])
    for half in range(2):
        cs = slice(half * 4, half * 4 + 4)
        eng[half].dma_start(out=wq_sb[:, cs, :],
                            in_=d_wq[cs].rearrange("c p n -> p c n"))
    for c in range(2, FC):
        eng[c % 2].dma_start(out=xT_sb[:, c, :], in_=d_xT[c])
    for half in range(2):
        cs = slice(half * 4, half * 4 + 4)
        eng[half].dma_start(out=wk_sb[:, cs, :],
                            in_=d_wk[cs].rearrange("c p n -> p c n"))
    nc.gpsimd.dma_start(out=wv_sb, in_=d_wv.rearrange("c p n -> p c n"))

    def unit_q(hp):
        qps = ps_mm.tile([P, SQ], F32, name="qps", tag="mm")
        for c in range(FC):
            nc.tensor.matmul(qps, lhsT=wq_sb[:, c, hp * P : (hp + 1) * P],
                             rhs=xT_sb[:, c, 0:SQ],
                             start=(c == 0), stop=(c == FC - 1))
        nc.vector.tensor_copy(qT_hp[hp], qps)

    def unit_k(hp, n):
        kps = ps_mm.tile([P, 512], F32, name="kps", tag="mm")
        for c in range(FC):
            nc.tensor.matmul(kps, lhsT=wk_sb[:, c, hp * P : (hp + 1) * P],
                             rhs=xT_sb[:, c, n * 512 : (n + 1) * 512],
                             start=(c == 0), stop=(c == FC - 1))
        nc.vector.tensor_copy(kT_hp[hp][:, n * 512 : (n + 1) * 512], kps)

    def unit_v(g, t):
        vps = ps_mm.tile([P, 512], F32, name="vps", tag="mm")
        for c in range(FC):
            nc.tensor.matmul(vps, lhsT=xT_sb[:, c, t * P : (t + 1) * P],
                             rhs=wv_sb[:, c, g * 512 : (g + 1) * 512],
                             start=(c == 0), stop=(c == FC - 1))
        nc.vector.tensor_copy(v_g[g][:, t, :, 0:HD],
                              vps.rearrange("p (h d) -> p h d", d=HD))

    # prefix: everything attention heads 0/1 need
    unit_q(0)
    for n in range(S // 512):
        unit_k(0, n)
    for t in range(KC):
        unit_v(0, t)

    units = deque()
    vg1 = deque((1, t) for t in range(KC))
    for hp in range(1, FC):
        units.append(("q", hp, 0))
        for n in range(S // 512):
            units.append(("k", hp, n))
        for _ in range(3):
            if vg1:
                units.append(("v", *vg1.popleft()))
    while vg1:
        units.append(("v", *vg1.popleft()))

    def emit_unit():
        if not units:
            return
        kind, a, b = units.popleft()
        if kind == "q":
            unit_q(a)
        elif kind == "k":
            unit_k(a, b)
        else:
            unit_v(a, b)

    # ---------------- attention (projection tail interleaved) ----------------
    CG = 2
    late = {}
    for h in range(NH):
        hp, hr = h // 2, (h % 2) * HD
        if h == FC:
            # all projection units have been emitted; free their SBUF and
            # bring in the dense/LN-phase tiles
            assert not units
            proj_ctx.close()
            late_pool = ctx.enter_context(tc.tile_pool(name="late", bufs=1))
            late["wd"] = late_pool.tile([P, FC, H], BF16, name="wd_sb")
            late["rec"] = late_pool.tile([NH, SQ], F32, name="rec_all")
            late["sel"] = late_pool.tile([NH, FC, P], F32, name="sel")
            nc.gpsimd.dma_start(out=late["sel"], in_=d_sel)
            nc.gpsimd.dma_start(out=late["wd"], in_=d_wd.rearrange("c p n -> p c n"))
        ctx_ps = ps_c.tile([P, SQ], F32, name="ctx_ps")
        for tg in range(KC // CG):
            s_ps = ps_s.tile([P, CG, SQ], F32, name="s_ps")
            for j in range(CG):
                t = tg * CG + j
                nc.tensor.matmul(
                    s_ps[:, j, :],
                    lhsT=kT_hp[hp][hr : hr + HD, t * P : (t + 1) * P],
                    rhs=qT_hp[hp][hr : hr + HD, :],
                    start=True, stop=True,
                )
            emit_unit()
            emit_unit()
            eT = work.tile([P, CG, SQ], BF16, name="eT")
            nc.scalar.activation(eT, s_ps, mybir.ActivationFunctionType.Exp,
                                 scale=SCALE)
            pT = work.tile([P, CG, SQ], BF16, name="pT")
            nc.vector.tensor_mul(pT, eT, maskT_sb[:, tg * CG : (tg + 1) * CG, :])
            for j in range(CG):
                t = tg * CG + j
                nc.tensor.matmul(
                    ctx_ps[0 : HD + 1, :],
                    lhsT=v_g[h // 8][:, t, h % 8, :],
                    rhs=pT[:, j, :],
                    start=(t == 0), stop=(t == KC - 1),
                )
        # cheap drains; division happens per head-pair as soon as both done
        nc.vector.tensor_copy(ctxT_sb[hr : hr + HD, hp, :], ctx_ps[0:HD, :])
        stmp = work.tile([1, SQ], F32, name="stmp")
        nc.vector.tensor_copy(stmp, ctx_ps[HD : HD + 1, :])
        # DMA sidesteps the partition-base restriction on compute engines
        nc.sync.dma_start(out=sums16[h : h + 1, :], in_=stmp)
    # batched normalization: one reciprocal over all heads, then per-head-pair
    # PE broadcast (selector matmul stacks both heads' recips) + one mul
    wd_sb = late["wd"]
    rec_all = late["rec"]
    nc.vector.reciprocal(rec_all, sums16)
    for cc in range(FC):
        bc_ps = ps_mm.tile([P, SQ], F32, name="bc_ps", tag="mm")
        nc.tensor.matmul(bc_ps, lhsT=late["sel"][:, cc, :], rhs=rec_all,
                         start=True, stop=True)
        nc.vector.tensor_mul(ctxT_sb[:, cc, :], ctxT_sb[:, cc, :], bc_ps)

    # ---------------- phase 3: dense + residual + LayerNorm ----------------
    ln_pool = ctx.enter_context(tc.tile_pool(name="ln", bufs=2))
    gb_pool = ctx.enter_context(tc.tile_pool(name="gb", bufs=1))
    eps_t = gb_pool.tile([P, 1], F32)
    nc.vector.memset(eps_t, EPS)
    if affine:
        gamma_bc = gb_pool.tile([P, H], F32)
        beta_bc = gb_pool.tile([P, H], F32)
        nc.sync.dma_start(out=gamma_bc, in_=_bcast_ap(d_gamma, P))
        nc.sync.dma_start(out=beta_bc, in_=_bcast_ap(d_beta, P))

    for r in range(SQ // P):
        pre = ln_pool.tile([P, H], F32, name="pre")
        xres_t = ln_pool.tile([P, H], F32, name="xres_t")
        nc.sync.dma_start(out=xres_t, in_=d_xres[r])
        for nh2 in range(H // 512):
            dps = ps_mm.tile([P, 512], F32, name="dps", tag="mm")
            for cc in range(FC):
                nc.tensor.matmul(
                    dps,
                    lhsT=ctxT_sb[:, cc, r * P : (r + 1) * P],
                    rhs=wd_sb[:, cc, nh2 * 512 : (nh2 + 1) * 512],
                    start=(cc == 0),
                    stop=(cc == FC - 1),
                )
            nc.vector.tensor_add(pre[:, nh2 * 512 : (nh2 + 1) * 512], dps,
                                 xres_t[:, nh2 * 512 : (nh2 + 1) * 512])

        # LayerNorm over free dim (1024) via bn_stats on two 512 subgroups
        stats = ln_pool.tile([P, 2, 6], F32, name="stats")
        nc.vector.bn_stats(stats[:, 0, :], pre[:, 0:512])
        nc.vector.bn_stats(stats[:, 1, :], pre[:, 512:1024])
        mv = ln_pool.tile([P, 2], F32, name="mv")
        nc.vector.bn_aggr(mv, stats)
        std = ln_pool.tile([P, 1], F32, name="std")
        nc.scalar.activation(std, mv[:, 1:2], mybir.ActivationFunctionType.Sqrt,
                             bias=eps_t)
        rstd = ln_pool.tile([P, 1], F32, name="rstd")
        nc.vector.reciprocal(rstd, std)
        outv = ln_pool.tile([P, H], F32, name="outv")
        nc.vector.tensor_scalar(outv, pre, mv[:, 0:1], rstd,
                                mybir.AluOpType.subtract, mybir.AluOpType.mult)
        if affine:
            nc.vector.tensor_mul(outv, outv, gamma_bc)
            nc.vector.tensor_add(outv, outv, beta_bc)
        nc.sync.dma_start(out=d_out[r], in_=outv)


_nc_cache = {}


def _get_nc(affine):
    if affine not in _nc_cache:
        _nc_cache[affine] = _build_program(affine)
    return _nc_cache[affine]


def kernel(hidden_states, mask, Wq, Wk, Wv, Wd, bd, gamma, beta):
    global last_results
    hidden_states = np.asarray(hidden_states, dtype=np.float32)
    mask = np.asarray(mask)
    Wq = np.asarray(Wq, dtype=np.float32)
    Wk = np.asarray(Wk, dtype=np.float32)
    Wv = np.asarray(Wv, dtype=np.float32)
    Wd = np.asarray(Wd, dtype=np.float32)
    bd = np.asarray(bd, dtype=np.float32)
    gamma = np.asarray(gamma, dtype=np.float32)
    beta = np.asarray(beta, dtype=np.float32)

    affine = bool(np.any(gamma != 1.0) or np.any(beta != 0.0))
    nc = _get_nc(affine)

    sel_np = np.zeros((NH, FC, P), dtype=np.float32)
    for cc in range(FC):
        sel_np[2 * cc, cc, 0:HD] = 1.0
        sel_np[2 * cc + 1, cc, HD:P] = 1.0

    wqT = np.ascontiguousarray(Wq.T).astype(NP_BF16).reshape(FC, P, H)
    wkT = np.ascontiguousarray(Wk.T).astype(NP_BF16).reshape(FC, P, H)
    wvT = np.ascontiguousarray(Wv.T).astype(NP_BF16).reshape(FC, P, H)
    wdT = np.ascontiguousarray(Wd.T).astype(NP_BF16).reshape(FC, P, H)

    in_maps = []
    for c in range(NCORES):
        b, qi = c // 4, c % 4
        qs = qi * SQ
        # roll the kv axis so this core's own query rows are columns 0..SQ
        xT = np.roll(hidden_states[b].T, -qs, axis=1)
        xT = np.ascontiguousarray(xT).astype(NP_BF16).reshape(FC, P, S)
        maskT = np.roll(mask[b].T, -qs, axis=0)[:, qs : qs + SQ]
        maskT = np.ascontiguousarray(maskT).astype(NP_BF16).reshape(KC, P, SQ)
        xres = (hidden_states[b, qs : qs + SQ] + bd[None, :]).astype(np.float32)
        in_maps.append({
            "xT": xT,
            "wqT": wqT,
            "wkT": wkT,
            "wvT": wvT,
            "wdT": wdT,
            "maskT": maskT,
            "xres": np.ascontiguousarray(xres.reshape(SQ // P, P, H)),
            "gamma": gamma,
            "beta": beta,
            "sel": sel_np,
        })

    trace = os.environ.get("BASS_KERNEL_TRACE", "0") == "1"
    res = run_bass_kernel_spmd(
        nc, in_maps, core_ids=list(range(NCORES)), trace=trace
    )
    last_results = res

    out = np.empty((B, S, H), dtype=np.float32)
    for c in range(NCORES):
        b, qi = c // 4, c % 4
        out[b, qi * SQ : (qi + 1) * SQ] = res.results[c]["out"].reshape(SQ, H)
    return out
